# revision 31
# baseline (speedup 1.0000x reference)
"""Multi-head self-attention (B=2, T=2048, D=1024, H=16) on 8 TRN2 NeuronCores.

Sharding: core c -> (b = c // 4, head-group hg = c % 4); each core computes the
full causal attention + partial output projection for its 4 heads of one batch
element.  Host pre-transposes x, pre-slices Wqkv / Wout per head group, and
sums the 4 fp16 partial projections per batch element (+ bout) at the end.

Single software-pipelined emission (vs the old 3-phase structure):
  A1: q/k projection chains for pair 0 (heads 0,1) only -> qT0/kT0 ready ~11us.
  A2: V chains + pair-1 q/k chains (PE fillers) woven with pair-0 score
      matmuls + exp -> pT0 tiles persisted in SBUF.  ScalarE exp stream starts
      ~12us instead of ~59us.
  B:  per i-tile: pair-1 scores/exp/AV + pair-0 AV (from stored pT0) + the
      previous i-tile's output projection, all interleaved so the in-order PE
      queue never sits behind a not-ready instruction.
Other changes vs baseline: causal N-slicing of diagonal score/AV matmuls
(saves ~15% of attention matmul cols), softmax reciprocal broadcast via two
K=1 PE matmuls (no DRAM round-trip), normalization folded into merged before
the output projection, fp16 partial outputs (halves the output DMA tail).
"""

import math
from contextlib import ExitStack

import numpy as np
import ml_dtypes

import concourse.bass as bass
import concourse.bacc as bacc_mod
import concourse.mybir as mybir
import concourse.tile as tile

FP32 = mybir.dt.float32
FP32R = mybir.dt.float32r
FP16 = mybir.dt.float16
BF16 = mybir.dt.bfloat16
AF = mybir.ActivationFunctionType
ALU = mybir.AluOpType

B, T, D, H = 2, 2048, 1024, 16
Dh = D // H          # 64
NCORES = 8
HPC = 4              # heads per core
NPAIR = HPC // 2     # head pairs (2 heads share a 128-partition block)
IT = T // 512        # 4 query tiles of 512
JB = T // 128        # 16 key blocks of 128
KO = D // 128        # 8 contraction blocks for the projections
SCALE = 1.0 / math.sqrt(Dh)


def build_program(compile=True):
    nc = bacc_mod.Bacc()

    xT = nc.declare_dram_parameter("xT", [D, T], BF16, isOutput=False)
    # wqk col blocks (cb): 0=q-pair0, 1=k-pair0, 2=q-pair1, 3=k-pair1
    wqk = nc.declare_dram_parameter("wqk", [128, KO, 4 * 128], BF16,
                                    isOutput=False)
    wv = nc.declare_dram_parameter("wv", [128, KO, HPC * Dh], BF16,
                                   isOutput=False)
    wout = nc.declare_dram_parameter("wout", [128, 2, D], BF16, isOutput=False)
    # consts: [tri 128 | unused 256 | ones 64]
    consts = nc.declare_dram_parameter("consts", [128, 448], BF16,
                                       isOutput=False)
    out = nc.declare_dram_parameter("out", [T, D], FP16, isOutput=True)

    xT_r = xT.rearrange("(o p) t -> p o t", p=128)

    with ExitStack() as ctx:
        tc = ctx.enter_context(tile.TileContext(nc))
        persist = ctx.enter_context(tc.tile_pool(name="persist", bufs=1))
        sbp = ctx.enter_context(tc.tile_pool(name="sbp", bufs=1))
        psp = ctx.enter_context(tc.tile_pool(name="psp", bufs=1, space="PSUM"))

        # ---------------- persistent tiles ----------------
        qkT = {}
        for nm in ("qT0", "kT0", "qT1", "kT1"):
            qkT[nm] = persist.tile([128, T], BF16, name=nm, tag=nm)
        V_aug = persist.tile([128, JB, HPC, 128], BF16, name="V_aug",
                             tag="V_aug")
        merged = [
            persist.tile([128, IT, 512], BF16, name=f"merged{p}",
                         tag=f"merged{p}")
            for p in range(NPAIR)
        ]
        wout_sb = persist.tile([128, 2, D], BF16, name="wout_sb",
                               tag="wout_sb")
        consts_sb = persist.tile([128, 448], BF16, name="consts_sb",
                                 tag="consts_sb")
        tri = consts_sb[:, 0:128]
        merged_flat = [m.rearrange("p a b -> p (a b)") for m in merged]
        recs = {}

        def ps_s_tile():
            return psp.tile([128, 2, 512], FP32, name="ps_s", tag="ps_s",
                            bufs=2)

        pools = {}

        def ctx0_tile():
            return pools["A"].tile([128, 2, 512], FP32, name="ctx0",
                                   tag="ctx0", bufs=1)

        def score(pair, it, jb, pT, ps2=None):
            """Row-packed score matmul pair + exp -> pT (bf16)."""
            kT_t = qkT[f"kT{pair}"]
            qT_t = qkT[f"qT{pair}"]
            jsl = slice(128 * jb, 128 * (jb + 1))
            q = jb - 4 * it
            c0 = 128 * q if q > 0 else 0
            i0 = 512 * it
            if ps2 is None:
                ps2 = ps_s_tile()
            for hl in range(2):
                rows = slice(64 * hl, 64 * (hl + 1))
                nc.tensor.matmul(
                    ps2[:, hl, c0:512],
                    lhsT=kT_t[rows, jsl],
                    rhs=qT_t[rows, i0 + c0: i0 + 512],
                    start=True, stop=True,
                )
            nc.scalar.activation(pT[:, :, c0:], ps2[:, :, c0:], AF.Exp,
                                 scale=SCALE)
            if q >= 0:
                for hl in range(2):
                    nc.vector.tensor_tensor(
                        out=pT[:, hl, c0: c0 + 128],
                        in0=pT[:, hl, c0: c0 + 128],
                        in1=tri[:],
                        op=ALU.mult,
                    )

        def av(pair, it, jb, pT, ctx_ps):
            q = jb - 4 * it
            c0 = 128 * q if q > 0 else 0
            njb = 4 * it + 4
            for hl in range(2):
                h = 2 * pair + hl
                nc.tensor.matmul(
                    ctx_ps[:, hl, c0:512],
                    lhsT=V_aug[:, jb, h, :],
                    rhs=pT[:, hl, c0:512],
                    start=(jb == 0), stop=(jb == njb - 1),
                )

        def sums_and_drain(it, pair, ctx_ps, seng=False):
            """1/softmax-sums + unnormalized ctx drain (frees ctx psum)."""
            r = sbp.tile([1, 2, 512], FP32, name="recs", tag=f"recs{pair}",
                         bufs=2)
            if seng:
                nc.scalar.copy(r[:], ctx_ps[64:65, :, :])
            else:
                nc.vector.tensor_copy(r[:], ctx_ps[64:65, :, :])
            nc.vector.reciprocal_approx_fast(r[:], r[:])
            recs[(it, pair)] = r
            nc.vector.tensor_copy(merged[pair][0:64, it], ctx_ps[0:64, 0, :])
            tmp = sbp.tile([64, 512], BF16, name="odd_tmp", tag="odd_tmp",
                           bufs=2)
            nc.scalar.copy(tmp[:], ctx_ps[0:64, 1, :])
            nc.sync.dma_start(merged[pair][64:128, it], tmp[:])

        def bc_normalize(it, pair):
            """Broadcast 1/sums across partitions (GpSimd), scale merged."""
            r = recs[(it, pair)]
            bc = sbp.tile([128, 2, 512], FP32, name="bc", tag="bc", bufs=2)
            nc.gpsimd.partition_broadcast(bc[:], r[:])
            for hl in range(2):
                rows = slice(64 * hl, 64 * (hl + 1))
                nc.vector.tensor_tensor(
                    out=merged[pair][rows, it],
                    in0=merged[pair][rows, it],
                    in1=bc[rows, hl, :],
                    op=ALU.mult,
                )

        def out_proj(tb, last=False):
            psos = [
                pools["B"].tile([128, 512], FP32, name="psos", tag="psos",
                                bufs=2)
                for _ in range(2)
            ]
            for pair in range(NPAIR):
                for et in range(2):
                    nc.tensor.matmul(
                        psos[et][:],
                        lhsT=merged_flat[pair][:, 128 * tb: 128 * (tb + 1)],
                        rhs=wout_sb[:, pair, 512 * et: 512 * (et + 1)],
                        start=(pair == 0), stop=(pair == NPAIR - 1),
                    )
            osb = sbp.tile([128, D], FP16, name="osb", tag="osb", bufs=3)
            if last:
                nc.scalar.copy(osb[:, 0:512], psos[0][:])
            else:
                nc.vector.tensor_copy(osb[:, 0:512], psos[0][:])
            nc.vector.tensor_copy(osb[:, 512:1024], psos[1][:])
            nc.sync.dma_start(out[128 * tb: 128 * (tb + 1), :], osb[:])

        # ---------------- phase A (+ pair-0 attention pipelined) --------
        with tc.tile_pool(name="poolA_sb", bufs=1) as pa_sb:
            xT_sb = pa_sb.tile([128, KO, T], BF16, name="xT_sb", tag="xT_sb")
            wqk_sb = pa_sb.tile([128, KO, 4 * 128], BF16, name="wqk_sb",
                                tag="wqk_sb")
            wv_sb = pa_sb.tile([128, KO, HPC * Dh], BF16, name="wv_sb",
                               tag="wv_sb")

            # Descriptor issue is ~0.4us each and serial per engine, so the
            # critical first tiles (pair-0 weights + the xT quarters the it0
            # chains need) are split fine and issued from two engines in
            # parallel; later bulk follows.
            nc.sync.dma_start(consts_sb[:], consts[:])
            for o in range(KO):
                nc.sync.dma_start(wqk_sb[:, o, 0:256], wqk[:, o, 0:256])
            nc.sync.dma_start(wv_sb[:], wv[:])
            for o in range(KO):
                nc.gpsimd.dma_start(xT_sb[:, o, 0:512], xT_r[:, o, 0:512])
            for o in range(KO):
                nc.gpsimd.dma_start(xT_sb[:, o, 512:1024],
                                    xT_r[:, o, 512:1024])
            for o in range(KO):
                nc.sync.dma_start(xT_sb[:, o, 1024:1536],
                                  xT_r[:, o, 1024:1536])
            for o in range(KO):
                nc.sync.dma_start(wqk_sb[:, o, 256:512], wqk[:, o, 256:512])
            for o in range(KO):
                nc.sync.dma_start(xT_sb[:, o, 1536:2048],
                                  xT_r[:, o, 1536:2048])
            nc.sync.dma_start(wout_sb[:], wout[:])
            nc.vector.tensor_copy(
                V_aug[:, :, :, 64:],
                consts_sb[:, None, None, 384:448].to_broadcast(
                    (128, JB, HPC, 64)),
            )

            with tc.tile_pool(name="poolA_ps", bufs=1, space="PSUM") as pa_ps:
                pools["A"] = pa_ps

                def qk_chain(cb, it, drain_eng):
                    dest = qkT[("qT0", "kT0", "qT1", "kT1")[cb]]
                    ps = pa_ps.tile([128, 512], FP32, name="chain",
                                    tag="chain", bufs=2)
                    for o in range(KO):
                        nc.tensor.matmul(
                            ps[:],
                            lhsT=wqk_sb[:, o, 128 * cb: 128 * (cb + 1)],
                            rhs=xT_sb[:, o, 512 * it: 512 * (it + 1)],
                            start=(o == 0), stop=(o == KO - 1),
                        )
                    if drain_eng == "s":
                        nc.scalar.copy(dest[:, 512 * it: 512 * (it + 1)],
                                       ps[:])
                    else:
                        nc.vector.tensor_copy(
                            dest[:, 512 * it: 512 * (it + 1)], ps[:])

                def v_chain(tb):
                    psv = pa_ps.tile([128, HPC * Dh], FP32, name="chain",
                                     tag="chain", bufs=2)
                    for o in range(KO):
                        nc.tensor.matmul(
                            psv[:],
                            lhsT=xT_sb[:, o, 128 * tb: 128 * (tb + 1)],
                            rhs=wv_sb[:, o],
                            start=(o == 0), stop=(o == KO - 1),
                        )
                    nc.vector.tensor_copy(
                        V_aug[:, tb, :, 0:64],
                        psv[:].rearrange("p (h d) -> p h d", h=HPC),
                    )

                # pair-0 it0/it1 q/k chains up front (ScalarE starts early)
                qk_chain(0, 0, "s")
                qk_chain(1, 0, "s")
                qk_chain(0, 1, "s")
                qk_chain(1, 1, "s")

                # pair-0 attention stream woven with the remaining chains.
                # Every filler carries a deadline: the block index by which
                # it must be EMITTED so its consumer (score / lagged AV)
                # never precedes it in the in-order engine queues.
                blocks = [(it, jb) for it in range(IT)
                          for jb in range(4 * it + 4)]
                bidx = {b: n for n, b in enumerate(blocks)}

                def first_use_block(tb):
                    it_min = 0 if tb < 4 else (tb - 3 + 3) // 4
                    return bidx[(it_min, tb)] + 1

                fillers = []
                for tb in range(JB):
                    fillers.append((("v", tb), first_use_block(tb)))
                for it in (2, 3):
                    fillers.append(
                        (("qk", 0, it), bidx[(it, 0)] - 1))
                    fillers.append(
                        (("qk", 1, it), bidx[(it, 0)] - 1))
                pre_dl = {0: 15, 1: 19, 2: 27, 3: len(blocks) - 1}
                for cb in (2, 3):
                    for it in range(IT):
                        fillers.append((("qk", cb, it), pre_dl[it]))
                NF, NB = len(fillers), len(blocks)
                fillers.sort(key=lambda fd: fd[1])
                buckets = {}
                for i, (f, dl) in enumerate(fillers):
                    n = min(i * NB // NF, dl)
                    buckets.setdefault(n, []).append(f)

                def emit_filler(f):
                    if f[0] == "qk":
                        qk_chain(f[1], f[2], "v")
                    else:
                        v_chain(f[1])

                LAG0 = 2
                pend = []
                ctx0 = None

                def flush_one():
                    nonlocal ctx0
                    pit, pjb, ppT = pend.pop(0)
                    if pjb == 0:
                        ctx0 = ctx0_tile()
                    av(0, pit, pjb, ppT, ctx0)
                    if pjb == 4 * pit + 3:       # last block of this it
                        sums_and_drain(pit, 0, ctx0)
                        bc_normalize(pit, 0)

                # pair-1 it0/it1 scores+exp run in A2's tail (ScalarE has
                # ~20us of idle there); their pT persists into phase B
                pre_list = [(0, j) for j in range(4)] + \
                           [(1, j) for j in range(8)] + \
                           [(2, j) for j in range(12)]
                pre_at = len(blocks) - len(pre_list)
                pT1pre = {}
                for n, (it, jb) in enumerate(blocks):
                    pT = sbp.tile([128, 2, 512], BF16, name="pT0", tag="pT0",
                                  bufs=LAG0 + 2)
                    score(0, it, jb, pT)
                    for f in buckets.pop(n, ()):
                        emit_filler(f)
                    if len(pend) >= LAG0:
                        flush_one()
                    pend.append((it, jb, pT))
                    if n >= pre_at:
                        pit1, pjb1 = pre_list[n - pre_at]
                        pTp = sbp.tile([128, 2, 512], BF16,
                                       name=f"pT1p_{pit1}_{pjb1}",
                                       tag=f"pT1p_{pit1}_{pjb1}", bufs=1)
                        score(1, pit1, pjb1, pTp)
                        pT1pre[(pit1, pjb1)] = pTp
                for blist in buckets.values():
                    for f in blist:
                        emit_filler(f)
                while pend:
                    flush_one()

        # ---------------- phase B: pair-1 stream + out projection --------
        LAG = 3
        with (
            tc.tile_pool(name="poolB_ps", bufs=1, space="PSUM") as psb,
        ):
            pools["B"] = psb

            def ctx1_tile():
                return psb.tile([128, 2, 512], FP32, name="ctx1", tag="ctx1",
                                bufs=1)

            # it3 is the only exp-paced segment left; everything after
            # it is dense pre-scored AV + output-projection matmul work, so
            # the PE stays busy (and the HAM clock warm) to the very end.
            it = 3
            njb = 16
            ctx1 = ctx1_tile()
            ring = {}
            for jb in range(njb):
                pT = sbp.tile([128, 2, 512], BF16, name="pT1", tag="pT1",
                              bufs=LAG + 2)
                score(1, it, jb, pT)
                ring[jb] = pT
                if jb >= LAG:
                    av(1, it, jb - LAG, ring.pop(jb - LAG), ctx1)
            for j2 in range(njb - LAG, njb):
                av(1, it, j2, ring.pop(j2), ctx1)
            sums_and_drain(3, 1, ctx1, seng=True)
            # it2: 12 AVs woven with it3's output tiles
            ctxp = ps_s_tile()
            bc_normalize(3, 1)
            for jb in range(12):
                av(1, 2, jb, pT1pre[(2, jb)], ctxp)
                if 4 <= jb <= 7:
                    out_proj(12 + (jb - 4), last=True)  # it3 tiles
            sums_and_drain(2, 1, ctxp, seng=True)
            # it1: 8 AVs woven with it2's output tiles
            ctxp = ps_s_tile()
            bc_normalize(2, 1)
            for jb in range(8):
                av(1, 1, jb, pT1pre[(1, jb)], ctxp)
                if 2 <= jb <= 5:
                    out_proj(8 + (jb - 2), last=True)   # it2 tiles
            sums_and_drain(1, 1, ctxp, seng=True)
            # it0: 4 AVs, then it1's output tiles
            ctxp = ps_s_tile()
            bc_normalize(1, 1)
            for jb in range(4):
                av(1, 0, jb, pT1pre[(0, jb)], ctxp)
            # drain+normalize it0 immediately so its chain overlaps the
            # it1 output tiles instead of preceding the final four alone
            sums_and_drain(0, 1, ctxp, seng=True)
            bc_normalize(0, 1)
            for tb in range(4, 8):
                out_proj(tb, last=True)                 # it1 tiles
            for tb in range(0, 4):
                out_proj(tb, last=True)                 # it0 tiles

    if compile:
        nc.compile()
    return nc


_PROGRAM = None


def _get_program():
    global _PROGRAM
    if _PROGRAM is None:
        _PROGRAM = build_program()
    return _PROGRAM


def _consts():
    c = np.zeros((128, 448), ml_dtypes.bfloat16)
    dj = np.arange(128)[:, None]
    di = np.arange(128)[None, :]
    c[:, 0:128] = (dj <= di).astype(ml_dtypes.bfloat16)  # causal triangle
    c[0, 128:192] = 1.0    # sel_h0: ones on cols 0:64
    c[0, 320:384] = 1.0    # sel_h1: ones on cols 64:128
    c[:, 384:448] = 1.0    # ones columns for V_aug
    return c


def make_in_maps(x, Wqkv, Wout):
    in_maps = []
    for core in range(NCORES):
        b, hg = core // (NCORES // B), core % (NCORES // B)
        c0 = hg * HPC * Dh
        blocks = [
            Wqkv[:, c0: c0 + 128],                    # q pair0
            Wqkv[:, D + c0: D + c0 + 128],            # k pair0
            Wqkv[:, c0 + 128: c0 + 256],              # q pair1
            Wqkv[:, D + c0 + 128: D + c0 + 256],      # k pair1
        ]
        wqk_full = np.concatenate(blocks, axis=1).astype(ml_dtypes.bfloat16)
        wv_full = Wqkv[:, 2 * D + c0: 2 * D + c0 + HPC * Dh].astype(
            ml_dtypes.bfloat16)
        in_maps.append({
            "consts": _consts(),
            "xT": np.ascontiguousarray(x[b].T).astype(ml_dtypes.bfloat16),
            "wqk": np.ascontiguousarray(
                wqk_full.reshape(KO, 128, 4 * 128).transpose(1, 0, 2)),
            "wv": np.ascontiguousarray(
                wv_full.reshape(KO, 128, HPC * Dh).transpose(1, 0, 2)),
            "wout": np.ascontiguousarray(
                Wout[c0: c0 + HPC * Dh, :].astype(ml_dtypes.bfloat16)
                .reshape(2, 128, D).transpose(1, 0, 2)),
        })
    return in_maps


def kernel(x, causal_mask, key_padding_mask, Wqkv, bqkv, Wout, bout,
           _trace=False):
    from concourse.bass_utils import run_bass_kernel_spmd

    x = np.asarray(x, dtype=np.float32)
    Wqkv = np.asarray(Wqkv, dtype=np.float32)
    Wout = np.asarray(Wout, dtype=np.float32)
    bqkv = np.asarray(bqkv, dtype=np.float32)
    bout = np.asarray(bout, dtype=np.float32)
    if np.any(np.asarray(key_padding_mask)):
        raise NotImplementedError("key_padding_mask with padded keys")
    if np.any(bqkv):
        raise NotImplementedError("nonzero bqkv")

    nc = _get_program()
    in_maps = make_in_maps(x, Wqkv, Wout)
    res = run_bass_kernel_spmd(nc, in_maps, core_ids=list(range(NCORES)),
                               trace=_trace)
    G = NCORES // B
    outp = np.empty((B, T, D), dtype=np.float32)
    for b in range(B):
        acc = res.results[b * G]["out"].astype(np.float32)
        for hg in range(1, G):
            acc = acc + res.results[b * G + hg]["out"].astype(np.float32)
        outp[b] = acc + bout
    kernel.last_exec_time_ns = res.exec_time_ns
    return outp


# revision 33
# speedup vs baseline: 1.0261x; 1.0261x over previous
"""Multi-head self-attention (B=2, T=2048, D=1024, H=16) on 8 TRN2 NeuronCores.

Sharding: core c -> (b = c // 4, head-group hg = c % 4); each core computes the
full causal attention + partial output projection for its 4 heads of one batch
element.  Host pre-transposes x, pre-slices Wqkv / Wout per head group, and
sums the 4 fp16 partial projections per batch element (+ bout) at the end.

Single software-pipelined emission (vs the old 3-phase structure):
  A1: q/k projection chains for pair 0 (heads 0,1) only -> qT0/kT0 ready ~11us.
  A2: V chains + pair-1 q/k chains (PE fillers) woven with pair-0 score
      matmuls + exp -> pT0 tiles persisted in SBUF.  ScalarE exp stream starts
      ~12us instead of ~59us.
  B:  per i-tile: pair-1 scores/exp/AV + pair-0 AV (from stored pT0) + the
      previous i-tile's output projection, all interleaved so the in-order PE
      queue never sits behind a not-ready instruction.
Other changes vs baseline: causal N-slicing of diagonal score/AV matmuls
(saves ~15% of attention matmul cols), softmax reciprocal broadcast via two
K=1 PE matmuls (no DRAM round-trip), normalization folded into merged before
the output projection, fp16 partial outputs (halves the output DMA tail).
"""

import math
from contextlib import ExitStack

import numpy as np
import ml_dtypes

import concourse.bass as bass
import concourse.bacc as bacc_mod
import concourse.mybir as mybir
import concourse.tile as tile

FP32 = mybir.dt.float32
FP32R = mybir.dt.float32r
FP16 = mybir.dt.float16
BF16 = mybir.dt.bfloat16
AF = mybir.ActivationFunctionType
ALU = mybir.AluOpType

B, T, D, H = 2, 2048, 1024, 16
Dh = D // H          # 64
NCORES = 8
HPC = 4              # heads per core
NPAIR = HPC // 2     # head pairs (2 heads share a 128-partition block)
IT = T // 512        # 4 query tiles of 512
JB = T // 128        # 16 key blocks of 128
KO = D // 128        # 8 contraction blocks for the projections
SCALE = 1.0 / math.sqrt(Dh)


def build_program(compile=True):
    nc = bacc_mod.Bacc()

    xT = nc.declare_dram_parameter("xT", [D, T], BF16, isOutput=False)
    # wqk col blocks (cb): 0=q-pair0, 1=k-pair0, 2=q-pair1, 3=k-pair1
    wqk = nc.declare_dram_parameter("wqk", [128, KO, 4 * 128], BF16,
                                    isOutput=False)
    wv = nc.declare_dram_parameter("wv", [128, KO, HPC * Dh], BF16,
                                   isOutput=False)
    wout = nc.declare_dram_parameter("wout", [128, 2, D], BF16, isOutput=False)
    # consts: [tri 128 | unused 256 | ones 64]
    consts = nc.declare_dram_parameter("consts", [128, 448], BF16,
                                       isOutput=False)
    out = nc.declare_dram_parameter("out", [T, D], FP16, isOutput=True)

    xT_r = xT.rearrange("(o p) t -> p o t", p=128)

    with ExitStack() as ctx:
        tc = ctx.enter_context(tile.TileContext(nc))
        persist = ctx.enter_context(tc.tile_pool(name="persist", bufs=1))
        sbp = ctx.enter_context(tc.tile_pool(name="sbp", bufs=1))
        psp = ctx.enter_context(tc.tile_pool(name="psp", bufs=1, space="PSUM"))

        # ---------------- persistent tiles ----------------
        qkT = {}
        for nm in ("qT0", "kT0", "qT1", "kT1"):
            qkT[nm] = persist.tile([128, T], BF16, name=nm, tag=nm)
        V_aug = persist.tile([128, JB, HPC, 128], BF16, name="V_aug",
                             tag="V_aug")
        merged = [
            persist.tile([128, IT, 512], BF16, name=f"merged{p}",
                         tag=f"merged{p}")
            for p in range(NPAIR)
        ]
        wout_sb = persist.tile([128, 2, D], BF16, name="wout_sb",
                               tag="wout_sb")
        consts_sb = persist.tile([128, 448], BF16, name="consts_sb",
                                 tag="consts_sb")
        tri = consts_sb[:, 0:128]
        merged_flat = [m.rearrange("p a b -> p (a b)") for m in merged]
        recs = {}

        def ps_s_tile():
            return psp.tile([128, 2, 512], FP32, name="ps_s", tag="ps_s",
                            bufs=2)

        pools = {}

        def ctx0_tile():
            return pools["A"].tile([128, 2, 512], FP32, name="ctx0",
                                   tag="ctx0", bufs=1)

        def score(pair, it, jb, pT, ps2=None):
            """Row-packed score matmul pair + exp -> pT (bf16)."""
            kT_t = qkT[f"kT{pair}"]
            qT_t = qkT[f"qT{pair}"]
            jsl = slice(128 * jb, 128 * (jb + 1))
            q = jb - 4 * it
            c0 = 128 * q if q > 0 else 0
            i0 = 512 * it
            if ps2 is None:
                ps2 = ps_s_tile()
            for hl in range(2):
                rows = slice(64 * hl, 64 * (hl + 1))
                nc.tensor.matmul(
                    ps2[:, hl, c0:512],
                    lhsT=kT_t[rows, jsl],
                    rhs=qT_t[rows, i0 + c0: i0 + 512],
                    start=True, stop=True,
                )
            nc.scalar.activation(pT[:, :, c0:], ps2[:, :, c0:], AF.Exp,
                                 scale=SCALE)
            if q >= 0:
                for hl in range(2):
                    nc.vector.tensor_tensor(
                        out=pT[:, hl, c0: c0 + 128],
                        in0=pT[:, hl, c0: c0 + 128],
                        in1=tri[:],
                        op=ALU.mult,
                    )

        def av(pair, it, jb, pT, ctx_ps):
            q = jb - 4 * it
            c0 = 128 * q if q > 0 else 0
            njb = 4 * it + 4
            for hl in range(2):
                h = 2 * pair + hl
                nc.tensor.matmul(
                    ctx_ps[:, hl, c0:512],
                    lhsT=V_aug[:, jb, h, :],
                    rhs=pT[:, hl, c0:512],
                    start=(jb == 0), stop=(jb == njb - 1),
                )

        def sums_and_drain(it, pair, ctx_ps, seng=False):
            """1/softmax-sums + unnormalized ctx drain (frees ctx psum)."""
            r = sbp.tile([1, 2, 512], FP32, name="recs", tag=f"recs{pair}",
                         bufs=2)
            if seng:
                nc.scalar.copy(r[:], ctx_ps[64:65, :, :])
            else:
                nc.vector.tensor_copy(r[:], ctx_ps[64:65, :, :])
            nc.vector.reciprocal_approx_fast(r[:], r[:])
            recs[(it, pair)] = r
            nc.vector.tensor_copy(merged[pair][0:64, it], ctx_ps[0:64, 0, :])
            tmp = sbp.tile([64, 512], BF16, name="odd_tmp", tag="odd_tmp",
                           bufs=2)
            nc.scalar.copy(tmp[:], ctx_ps[0:64, 1, :])
            nc.sync.dma_start(merged[pair][64:128, it], tmp[:])

        def bc_normalize(it, pair):
            """Broadcast 1/sums across partitions (GpSimd), scale merged."""
            r = recs[(it, pair)]
            bc = sbp.tile([128, 2, 512], FP32, name="bc", tag="bc", bufs=2)
            nc.gpsimd.partition_broadcast(bc[:], r[:])
            for hl in range(2):
                rows = slice(64 * hl, 64 * (hl + 1))
                nc.vector.tensor_tensor(
                    out=merged[pair][rows, it],
                    in0=merged[pair][rows, it],
                    in1=bc[rows, hl, :],
                    op=ALU.mult,
                )

        def out_proj(tb, last=False):
            psos = [
                pools["B"].tile([128, 512], FP32, name="psos", tag="psos",
                                bufs=2)
                for _ in range(2)
            ]
            for pair in range(NPAIR):
                for et in range(2):
                    nc.tensor.matmul(
                        psos[et][:],
                        lhsT=merged_flat[pair][:, 128 * tb: 128 * (tb + 1)],
                        rhs=wout_sb[:, pair, 512 * et: 512 * (et + 1)],
                        start=(pair == 0), stop=(pair == NPAIR - 1),
                    )
            osb = sbp.tile([128, D], FP16, name="osb", tag="osb", bufs=3)
            if last:
                nc.scalar.copy(osb[:, 0:512], psos[0][:])
            else:
                nc.vector.tensor_copy(osb[:, 0:512], psos[0][:])
            nc.vector.tensor_copy(osb[:, 512:1024], psos[1][:])
            nc.sync.dma_start(out[128 * tb: 128 * (tb + 1), :], osb[:])

        # ---------------- phase A (+ pair-0 attention pipelined) --------
        with tc.tile_pool(name="poolA_sb", bufs=1) as pa_sb:
            xT_sb = pa_sb.tile([128, KO, T], BF16, name="xT_sb", tag="xT_sb")
            wqk_sb = pa_sb.tile([128, KO, 4 * 128], BF16, name="wqk_sb",
                                tag="wqk_sb")
            wv_sb = pa_sb.tile([128, KO, HPC * Dh], BF16, name="wv_sb",
                               tag="wv_sb")

            # Descriptor issue is ~0.4us each and serial per engine, so the
            # critical first tiles (pair-0 weights + the xT quarters the it0
            # chains need) are split fine and issued from two engines in
            # parallel; later bulk follows.
            nc.sync.dma_start(consts_sb[:], consts[:])
            for o in range(KO):
                nc.sync.dma_start(wqk_sb[:, o, 0:256], wqk[:, o, 0:256])
            nc.sync.dma_start(wv_sb[:], wv[:])
            for o in range(KO):
                nc.gpsimd.dma_start(xT_sb[:, o, 0:512], xT_r[:, o, 0:512])
            for o in range(KO):
                nc.gpsimd.dma_start(xT_sb[:, o, 512:1024],
                                    xT_r[:, o, 512:1024])
            for o in range(KO):
                nc.sync.dma_start(xT_sb[:, o, 1024:1536],
                                  xT_r[:, o, 1024:1536])
            for o in range(KO):
                nc.sync.dma_start(wqk_sb[:, o, 256:512], wqk[:, o, 256:512])
            for o in range(KO):
                nc.sync.dma_start(xT_sb[:, o, 1536:2048],
                                  xT_r[:, o, 1536:2048])
            nc.sync.dma_start(wout_sb[:], wout[:])
            nc.vector.tensor_copy(
                V_aug[:, :, :, 64:],
                consts_sb[:, None, None, 384:448].to_broadcast(
                    (128, JB, HPC, 64)),
            )

            with tc.tile_pool(name="poolA_ps", bufs=1, space="PSUM") as pa_ps:
                pools["A"] = pa_ps

                def qk_chain(cb, it, drain_eng):
                    dest = qkT[("qT0", "kT0", "qT1", "kT1")[cb]]
                    ps = pa_ps.tile([128, 512], FP32, name="chain",
                                    tag="chain", bufs=2)
                    for o in range(KO):
                        nc.tensor.matmul(
                            ps[:],
                            lhsT=wqk_sb[:, o, 128 * cb: 128 * (cb + 1)],
                            rhs=xT_sb[:, o, 512 * it: 512 * (it + 1)],
                            start=(o == 0), stop=(o == KO - 1),
                        )
                    if drain_eng == "s":
                        nc.scalar.copy(dest[:, 512 * it: 512 * (it + 1)],
                                       ps[:])
                    else:
                        nc.vector.tensor_copy(
                            dest[:, 512 * it: 512 * (it + 1)], ps[:])

                def v_chain(tb):
                    psv = pa_ps.tile([128, HPC * Dh], FP32, name="chain",
                                     tag="chain", bufs=2)
                    for o in range(KO):
                        nc.tensor.matmul(
                            psv[:],
                            lhsT=xT_sb[:, o, 128 * tb: 128 * (tb + 1)],
                            rhs=wv_sb[:, o],
                            start=(o == 0), stop=(o == KO - 1),
                        )
                    nc.vector.tensor_copy(
                        V_aug[:, tb, :, 0:64],
                        psv[:].rearrange("p (h d) -> p h d", h=HPC),
                    )

                # pair-0 it0 q/k chains up front (ScalarE starts early);
                # the it1 chains wait on later xT quarters, so they ride as
                # deadline-fillers instead of blocking the first scores
                qk_chain(0, 0, "s")
                qk_chain(1, 0, "s")

                # pair-0 attention stream woven with the remaining chains.
                # Every filler carries a deadline: the block index by which
                # it must be EMITTED so its consumer (score / lagged AV)
                # never precedes it in the in-order engine queues.
                blocks = [(it, jb) for it in range(IT)
                          for jb in range(4 * it + 4)]
                bidx = {b: n for n, b in enumerate(blocks)}

                def first_use_block(tb):
                    it_min = 0 if tb < 4 else (tb - 3 + 3) // 4
                    return bidx[(it_min, tb)] + 1

                fillers = []
                fillers.append((("qk", 0, 1), 3))
                fillers.append((("qk", 1, 1), 3))
                for tb in range(JB):
                    fillers.append((("v", tb), first_use_block(tb)))
                for it in (2, 3):
                    fillers.append(
                        (("qk", 0, it), bidx[(it, 0)] - 1))
                    fillers.append(
                        (("qk", 1, it), bidx[(it, 0)] - 1))
                pre_dl = {0: 15, 1: 19, 2: 27, 3: len(blocks) - 1}
                for cb in (2, 3):
                    for it in range(IT):
                        fillers.append((("qk", cb, it), pre_dl[it]))
                NF, NB = len(fillers), len(blocks)
                fillers.sort(key=lambda fd: fd[1])
                buckets = {}
                for i, (f, dl) in enumerate(fillers):
                    n = min(i * NB // NF, dl)
                    buckets.setdefault(n, []).append(f)

                def emit_filler(f):
                    if f[0] == "qk":
                        qk_chain(f[1], f[2], "v")
                    else:
                        v_chain(f[1])

                LAG0 = 2
                pend = []
                ctx0 = None

                def flush_one():
                    nonlocal ctx0
                    pit, pjb, ppT = pend.pop(0)
                    if pjb == 0:
                        ctx0 = ctx0_tile()
                    av(0, pit, pjb, ppT, ctx0)
                    if pjb == 4 * pit + 3:       # last block of this it
                        sums_and_drain(pit, 0, ctx0)
                        bc_normalize(pit, 0)

                # pair-1 it0/it1 scores+exp run in A2's tail (ScalarE has
                # ~20us of idle there); their pT persists into phase B
                pre_list = [(0, j) for j in range(4)] + \
                           [(1, j) for j in range(8)] + \
                           [(2, j) for j in range(12)]
                pre_at = len(blocks) - len(pre_list)
                pT1pre = {}
                for n, (it, jb) in enumerate(blocks):
                    pT = sbp.tile([128, 2, 512], BF16, name="pT0", tag="pT0",
                                  bufs=LAG0 + 2)
                    score(0, it, jb, pT)
                    for f in buckets.pop(n, ()):
                        emit_filler(f)
                    if len(pend) >= LAG0:
                        flush_one()
                    pend.append((it, jb, pT))
                    if n >= pre_at:
                        pit1, pjb1 = pre_list[n - pre_at]
                        pTp = sbp.tile([128, 2, 512], BF16,
                                       name=f"pT1p_{pit1}_{pjb1}",
                                       tag=f"pT1p_{pit1}_{pjb1}", bufs=1)
                        score(1, pit1, pjb1, pTp)
                        pT1pre[(pit1, pjb1)] = pTp
                for blist in buckets.values():
                    for f in blist:
                        emit_filler(f)
                while pend:
                    flush_one()

        # ---------------- phase B: pair-1 stream + out projection --------
        LAG = 3
        with (
            tc.tile_pool(name="poolB_ps", bufs=1, space="PSUM") as psb,
        ):
            pools["B"] = psb

            def ctx1_tile():
                return psb.tile([128, 2, 512], FP32, name="ctx1", tag="ctx1",
                                bufs=1)

            # it3 is the only exp-paced segment left; everything after
            # it is dense pre-scored AV + output-projection matmul work, so
            # the PE stays busy (and the HAM clock warm) to the very end.
            it = 3
            njb = 16
            ctx1 = ctx1_tile()
            ring = {}
            for jb in range(njb):
                pT = sbp.tile([128, 2, 512], BF16, name="pT1", tag="pT1",
                              bufs=LAG + 2)
                score(1, it, jb, pT)
                ring[jb] = pT
                if jb >= LAG:
                    av(1, it, jb - LAG, ring.pop(jb - LAG), ctx1)
            for j2 in range(njb - LAG, njb):
                av(1, it, j2, ring.pop(j2), ctx1)
            sums_and_drain(3, 1, ctx1, seng=True)
            # it2: 12 AVs woven with it3's output tiles
            ctxp = ps_s_tile()
            bc_normalize(3, 1)
            for jb in range(12):
                av(1, 2, jb, pT1pre[(2, jb)], ctxp)
                if 3 <= jb <= 6:
                    out_proj(12 + (jb - 3), last=True)  # it3 tiles
            sums_and_drain(2, 1, ctxp, seng=True)
            # it1: 8 AVs woven with it2's output tiles
            ctxp = ps_s_tile()
            bc_normalize(2, 1)
            for jb in range(8):
                av(1, 1, jb, pT1pre[(1, jb)], ctxp)
                if 2 <= jb <= 5:
                    out_proj(8 + (jb - 2), last=True)   # it2 tiles
            sums_and_drain(1, 1, ctxp, seng=True)
            # it0: 4 AVs, then it1's output tiles
            ctxp = ps_s_tile()
            bc_normalize(1, 1)
            for jb in range(4):
                av(1, 0, jb, pT1pre[(0, jb)], ctxp)
            for tb in range(4, 8):
                out_proj(tb, last=True)                 # it1 tiles
            sums_and_drain(0, 1, ctxp, seng=True)
            bc_normalize(0, 1)
            for tb in range(0, 4):
                out_proj(tb, last=True)                 # it0 tiles

    if compile:
        nc.compile()
    return nc


_PROGRAM = None


def _get_program():
    global _PROGRAM
    if _PROGRAM is None:
        _PROGRAM = build_program()
    return _PROGRAM


def _consts():
    c = np.zeros((128, 448), ml_dtypes.bfloat16)
    dj = np.arange(128)[:, None]
    di = np.arange(128)[None, :]
    c[:, 0:128] = (dj <= di).astype(ml_dtypes.bfloat16)  # causal triangle
    c[0, 128:192] = 1.0    # sel_h0: ones on cols 0:64
    c[0, 320:384] = 1.0    # sel_h1: ones on cols 64:128
    c[:, 384:448] = 1.0    # ones columns for V_aug
    return c


def make_in_maps(x, Wqkv, Wout):
    in_maps = []
    for core in range(NCORES):
        b, hg = core // (NCORES // B), core % (NCORES // B)
        c0 = hg * HPC * Dh
        blocks = [
            Wqkv[:, c0: c0 + 128],                    # q pair0
            Wqkv[:, D + c0: D + c0 + 128],            # k pair0
            Wqkv[:, c0 + 128: c0 + 256],              # q pair1
            Wqkv[:, D + c0 + 128: D + c0 + 256],      # k pair1
        ]
        wqk_full = np.concatenate(blocks, axis=1).astype(ml_dtypes.bfloat16)
        wv_full = Wqkv[:, 2 * D + c0: 2 * D + c0 + HPC * Dh].astype(
            ml_dtypes.bfloat16)
        in_maps.append({
            "consts": _consts(),
            "xT": np.ascontiguousarray(x[b].T).astype(ml_dtypes.bfloat16),
            "wqk": np.ascontiguousarray(
                wqk_full.reshape(KO, 128, 4 * 128).transpose(1, 0, 2)),
            "wv": np.ascontiguousarray(
                wv_full.reshape(KO, 128, HPC * Dh).transpose(1, 0, 2)),
            "wout": np.ascontiguousarray(
                Wout[c0: c0 + HPC * Dh, :].astype(ml_dtypes.bfloat16)
                .reshape(2, 128, D).transpose(1, 0, 2)),
        })
    return in_maps


def kernel(x, causal_mask, key_padding_mask, Wqkv, bqkv, Wout, bout,
           _trace=False):
    from concourse.bass_utils import run_bass_kernel_spmd

    x = np.asarray(x, dtype=np.float32)
    Wqkv = np.asarray(Wqkv, dtype=np.float32)
    Wout = np.asarray(Wout, dtype=np.float32)
    bqkv = np.asarray(bqkv, dtype=np.float32)
    bout = np.asarray(bout, dtype=np.float32)
    if np.any(np.asarray(key_padding_mask)):
        raise NotImplementedError("key_padding_mask with padded keys")
    if np.any(bqkv):
        raise NotImplementedError("nonzero bqkv")

    nc = _get_program()
    in_maps = make_in_maps(x, Wqkv, Wout)
    res = run_bass_kernel_spmd(nc, in_maps, core_ids=list(range(NCORES)),
                               trace=_trace)
    G = NCORES // B
    outp = np.empty((B, T, D), dtype=np.float32)
    for b in range(B):
        acc = res.results[b * G]["out"].astype(np.float32)
        for hg in range(1, G):
            acc = acc + res.results[b * G + hg]["out"].astype(np.float32)
        outp[b] = acc + bout
    kernel.last_exec_time_ns = res.exec_time_ns
    return outp


# revision 34
# speedup vs baseline: 1.0309x; 1.0047x over previous
"""Multi-head self-attention (B=2, T=2048, D=1024, H=16) on 8 TRN2 NeuronCores.

Sharding: core c -> (b = c // 4, head-group hg = c % 4); each core computes the
full causal attention + partial output projection for its 4 heads of one batch
element.  Host pre-transposes x, pre-slices Wqkv / Wout per head group, and
sums the 4 fp16 partial projections per batch element (+ bout) at the end.

Single software-pipelined emission (vs the old 3-phase structure):
  A1: q/k projection chains for pair 0 (heads 0,1) only -> qT0/kT0 ready ~11us.
  A2: V chains + pair-1 q/k chains (PE fillers) woven with pair-0 score
      matmuls + exp -> pT0 tiles persisted in SBUF.  ScalarE exp stream starts
      ~12us instead of ~59us.
  B:  per i-tile: pair-1 scores/exp/AV + pair-0 AV (from stored pT0) + the
      previous i-tile's output projection, all interleaved so the in-order PE
      queue never sits behind a not-ready instruction.
Other changes vs baseline: causal N-slicing of diagonal score/AV matmuls
(saves ~15% of attention matmul cols), softmax reciprocal broadcast via two
K=1 PE matmuls (no DRAM round-trip), normalization folded into merged before
the output projection, fp16 partial outputs (halves the output DMA tail).
"""

import math
from contextlib import ExitStack

import numpy as np
import ml_dtypes

import concourse.bass as bass
import concourse.bacc as bacc_mod
import concourse.mybir as mybir
import concourse.tile as tile

FP32 = mybir.dt.float32
FP32R = mybir.dt.float32r
FP16 = mybir.dt.float16
BF16 = mybir.dt.bfloat16
AF = mybir.ActivationFunctionType
ALU = mybir.AluOpType

B, T, D, H = 2, 2048, 1024, 16
Dh = D // H          # 64
NCORES = 8
HPC = 4              # heads per core
NPAIR = HPC // 2     # head pairs (2 heads share a 128-partition block)
IT = T // 512        # 4 query tiles of 512
JB = T // 128        # 16 key blocks of 128
KO = D // 128        # 8 contraction blocks for the projections
SCALE = 1.0 / math.sqrt(Dh)


def build_program(compile=True):
    nc = bacc_mod.Bacc()

    xT = nc.declare_dram_parameter("xT", [D, T], BF16, isOutput=False)
    # wqk col blocks (cb): 0=q-pair0, 1=k-pair0, 2=q-pair1, 3=k-pair1
    wqk = nc.declare_dram_parameter("wqk", [128, KO, 4 * 128], BF16,
                                    isOutput=False)
    wv = nc.declare_dram_parameter("wv", [128, KO, HPC * Dh], BF16,
                                   isOutput=False)
    wout = nc.declare_dram_parameter("wout", [128, 2, D], BF16, isOutput=False)
    # consts: [tri 128 | unused 256 | ones 64]
    consts = nc.declare_dram_parameter("consts", [128, 448], BF16,
                                       isOutput=False)
    out = nc.declare_dram_parameter("out", [T, D], FP16, isOutput=True)

    xT_r = xT.rearrange("(o p) t -> p o t", p=128)

    with ExitStack() as ctx:
        tc = ctx.enter_context(tile.TileContext(nc))
        persist = ctx.enter_context(tc.tile_pool(name="persist", bufs=1))
        sbp = ctx.enter_context(tc.tile_pool(name="sbp", bufs=1))
        psp = ctx.enter_context(tc.tile_pool(name="psp", bufs=1, space="PSUM"))

        # ---------------- persistent tiles ----------------
        qkT = {}
        for nm in ("qT0", "kT0", "qT1", "kT1"):
            qkT[nm] = persist.tile([128, T], BF16, name=nm, tag=nm)
        V_aug = persist.tile([128, JB, HPC, 128], BF16, name="V_aug",
                             tag="V_aug")
        merged = [
            persist.tile([128, IT, 512], BF16, name=f"merged{p}",
                         tag=f"merged{p}")
            for p in range(NPAIR)
        ]
        wout_sb = persist.tile([128, 2, D], BF16, name="wout_sb",
                               tag="wout_sb")
        consts_sb = persist.tile([128, 448], BF16, name="consts_sb",
                                 tag="consts_sb")
        tri = consts_sb[:, 0:128]
        merged_flat = [m.rearrange("p a b -> p (a b)") for m in merged]
        recs = {}

        def ps_s_tile():
            return psp.tile([128, 2, 512], FP32, name="ps_s", tag="ps_s",
                            bufs=2)

        pools = {}

        def ctx0_tile():
            return pools["A"].tile([128, 2, 512], FP32, name="ctx0",
                                   tag="ctx0", bufs=1)

        def score(pair, it, jb, pT, ps2=None):
            """Row-packed score matmul pair + exp -> pT (bf16)."""
            kT_t = qkT[f"kT{pair}"]
            qT_t = qkT[f"qT{pair}"]
            jsl = slice(128 * jb, 128 * (jb + 1))
            q = jb - 4 * it
            c0 = 128 * q if q > 0 else 0
            i0 = 512 * it
            if ps2 is None:
                ps2 = ps_s_tile()
            for hl in range(2):
                rows = slice(64 * hl, 64 * (hl + 1))
                nc.tensor.matmul(
                    ps2[:, hl, c0:512],
                    lhsT=kT_t[rows, jsl],
                    rhs=qT_t[rows, i0 + c0: i0 + 512],
                    start=True, stop=True,
                )
            nc.scalar.activation(pT[:, :, c0:], ps2[:, :, c0:], AF.Exp,
                                 scale=SCALE)
            if q >= 0:
                for hl in range(2):
                    nc.vector.tensor_tensor(
                        out=pT[:, hl, c0: c0 + 128],
                        in0=pT[:, hl, c0: c0 + 128],
                        in1=tri[:],
                        op=ALU.mult,
                    )

        def av(pair, it, jb, pT, ctx_ps):
            q = jb - 4 * it
            c0 = 128 * q if q > 0 else 0
            njb = 4 * it + 4
            for hl in range(2):
                h = 2 * pair + hl
                nc.tensor.matmul(
                    ctx_ps[:, hl, c0:512],
                    lhsT=V_aug[:, jb, h, :],
                    rhs=pT[:, hl, c0:512],
                    start=(jb == 0), stop=(jb == njb - 1),
                )

        def sums_and_drain(it, pair, ctx_ps, seng=False):
            """1/softmax-sums + unnormalized ctx drain (frees ctx psum)."""
            r = sbp.tile([1, 2, 512], FP32, name="recs", tag=f"recs{pair}",
                         bufs=2)
            if seng:
                nc.scalar.copy(r[:], ctx_ps[64:65, :, :])
            else:
                nc.vector.tensor_copy(r[:], ctx_ps[64:65, :, :])
            nc.vector.reciprocal_approx_fast(r[:], r[:])
            recs[(it, pair)] = r
            nc.vector.tensor_copy(merged[pair][0:64, it], ctx_ps[0:64, 0, :])
            tmp = sbp.tile([64, 512], BF16, name="odd_tmp", tag="odd_tmp",
                           bufs=2)
            nc.scalar.copy(tmp[:], ctx_ps[0:64, 1, :])
            nc.sync.dma_start(merged[pair][64:128, it], tmp[:])

        def bc_normalize(it, pair):
            """Broadcast 1/sums across partitions (GpSimd), scale merged."""
            r = recs[(it, pair)]
            bc = sbp.tile([128, 2, 512], FP32, name="bc", tag="bc", bufs=2)
            nc.gpsimd.partition_broadcast(bc[:], r[:])
            for hl in range(2):
                rows = slice(64 * hl, 64 * (hl + 1))
                nc.vector.tensor_tensor(
                    out=merged[pair][rows, it],
                    in0=merged[pair][rows, it],
                    in1=bc[rows, hl, :],
                    op=ALU.mult,
                )

        def out_proj(tb, last=False):
            psos = [
                pools["B"].tile([128, 512], FP32, name="psos", tag="psos",
                                bufs=2)
                for _ in range(2)
            ]
            for pair in range(NPAIR):
                for et in range(2):
                    nc.tensor.matmul(
                        psos[et][:],
                        lhsT=merged_flat[pair][:, 128 * tb: 128 * (tb + 1)],
                        rhs=wout_sb[:, pair, 512 * et: 512 * (et + 1)],
                        start=(pair == 0), stop=(pair == NPAIR - 1),
                    )
            osb = sbp.tile([128, D], FP16, name="osb", tag="osb", bufs=3)
            if last:
                nc.scalar.copy(osb[:, 0:512], psos[0][:])
            else:
                nc.vector.tensor_copy(osb[:, 0:512], psos[0][:])
            nc.vector.tensor_copy(osb[:, 512:1024], psos[1][:])
            nc.sync.dma_start(out[128 * tb: 128 * (tb + 1), :], osb[:])

        # ---------------- phase A (+ pair-0 attention pipelined) --------
        with tc.tile_pool(name="poolA_sb", bufs=1) as pa_sb:
            xT_sb = pa_sb.tile([128, KO, T], BF16, name="xT_sb", tag="xT_sb")
            wqk_sb = pa_sb.tile([128, KO, 4 * 128], BF16, name="wqk_sb",
                                tag="wqk_sb")
            wv_sb = pa_sb.tile([128, KO, HPC * Dh], BF16, name="wv_sb",
                               tag="wv_sb")

            # Descriptor issue is ~0.4us each and serial per engine, so the
            # critical first tiles (pair-0 weights + the xT quarters the it0
            # chains need) are split fine and issued from two engines in
            # parallel; later bulk follows.
            for o in range(KO):
                nc.sync.dma_start(wqk_sb[:, o, 0:256], wqk[:, o, 0:256])
            nc.sync.dma_start(consts_sb[:], consts[:])
            nc.sync.dma_start(wv_sb[:], wv[:])
            for o in range(KO):
                nc.gpsimd.dma_start(xT_sb[:, o, 0:512], xT_r[:, o, 0:512])
            for o in range(KO):
                nc.gpsimd.dma_start(xT_sb[:, o, 512:1024],
                                    xT_r[:, o, 512:1024])
            for o in range(KO):
                nc.sync.dma_start(xT_sb[:, o, 1024:1536],
                                  xT_r[:, o, 1024:1536])
            for o in range(KO):
                nc.sync.dma_start(wqk_sb[:, o, 256:512], wqk[:, o, 256:512])
            for o in range(KO):
                nc.sync.dma_start(xT_sb[:, o, 1536:2048],
                                  xT_r[:, o, 1536:2048])
            nc.sync.dma_start(wout_sb[:], wout[:])
            nc.vector.tensor_copy(
                V_aug[:, :, :, 64:],
                consts_sb[:, None, None, 384:448].to_broadcast(
                    (128, JB, HPC, 64)),
            )

            with tc.tile_pool(name="poolA_ps", bufs=1, space="PSUM") as pa_ps:
                pools["A"] = pa_ps

                def qk_chain(cb, it, drain_eng):
                    dest = qkT[("qT0", "kT0", "qT1", "kT1")[cb]]
                    ps = pa_ps.tile([128, 512], FP32, name="chain",
                                    tag="chain", bufs=2)
                    for o in range(KO):
                        nc.tensor.matmul(
                            ps[:],
                            lhsT=wqk_sb[:, o, 128 * cb: 128 * (cb + 1)],
                            rhs=xT_sb[:, o, 512 * it: 512 * (it + 1)],
                            start=(o == 0), stop=(o == KO - 1),
                        )
                    if drain_eng == "s":
                        nc.scalar.copy(dest[:, 512 * it: 512 * (it + 1)],
                                       ps[:])
                    else:
                        nc.vector.tensor_copy(
                            dest[:, 512 * it: 512 * (it + 1)], ps[:])

                def v_chain(tb):
                    psv = pa_ps.tile([128, HPC * Dh], FP32, name="chain",
                                     tag="chain", bufs=2)
                    for o in range(KO):
                        nc.tensor.matmul(
                            psv[:],
                            lhsT=xT_sb[:, o, 128 * tb: 128 * (tb + 1)],
                            rhs=wv_sb[:, o],
                            start=(o == 0), stop=(o == KO - 1),
                        )
                    nc.vector.tensor_copy(
                        V_aug[:, tb, :, 0:64],
                        psv[:].rearrange("p (h d) -> p h d", h=HPC),
                    )

                # pair-0 it0 q/k chains up front (ScalarE starts early);
                # the it1 chains wait on later xT quarters, so they ride as
                # deadline-fillers instead of blocking the first scores
                qk_chain(0, 0, "s")
                qk_chain(1, 0, "s")

                # pair-0 attention stream woven with the remaining chains.
                # Every filler carries a deadline: the block index by which
                # it must be EMITTED so its consumer (score / lagged AV)
                # never precedes it in the in-order engine queues.
                blocks = [(it, jb) for it in range(IT)
                          for jb in range(4 * it + 4)]
                bidx = {b: n for n, b in enumerate(blocks)}

                def first_use_block(tb):
                    it_min = 0 if tb < 4 else (tb - 3 + 3) // 4
                    return bidx[(it_min, tb)] + 1

                fillers = []
                fillers.append((("qk", 0, 1), 3))
                fillers.append((("qk", 1, 1), 3))
                for tb in range(JB):
                    fillers.append((("v", tb), first_use_block(tb)))
                for it in (2, 3):
                    fillers.append(
                        (("qk", 0, it), bidx[(it, 0)] - 1))
                    fillers.append(
                        (("qk", 1, it), bidx[(it, 0)] - 1))
                pre_dl = {0: 15, 1: 19, 2: 27, 3: len(blocks) - 1}
                for cb in (2, 3):
                    for it in range(IT):
                        fillers.append((("qk", cb, it), pre_dl[it]))
                NF, NB = len(fillers), len(blocks)
                fillers.sort(key=lambda fd: fd[1])
                buckets = {}
                for i, (f, dl) in enumerate(fillers):
                    n = min(i * NB // NF, dl)
                    buckets.setdefault(n, []).append(f)

                def emit_filler(f):
                    if f[0] == "qk":
                        qk_chain(f[1], f[2], "v")
                    else:
                        v_chain(f[1])

                LAG0 = 3
                pend = []
                ctx0 = None

                def flush_one():
                    nonlocal ctx0
                    pit, pjb, ppT = pend.pop(0)
                    if pjb == 0:
                        ctx0 = ctx0_tile()
                    av(0, pit, pjb, ppT, ctx0)
                    if pjb == 4 * pit + 3:       # last block of this it
                        sums_and_drain(pit, 0, ctx0)
                        bc_normalize(pit, 0)

                # pair-1 it0/it1 scores+exp run in A2's tail (ScalarE has
                # ~20us of idle there); their pT persists into phase B
                pre_list = [(0, j) for j in range(4)] + \
                           [(1, j) for j in range(8)] + \
                           [(2, j) for j in range(12)]
                pre_at = len(blocks) - len(pre_list)
                pT1pre = {}
                for n, (it, jb) in enumerate(blocks):
                    pT = sbp.tile([128, 2, 512], BF16, name="pT0", tag="pT0",
                                  bufs=LAG0 + 2)
                    score(0, it, jb, pT)
                    for f in buckets.pop(n, ()):
                        emit_filler(f)
                    if len(pend) >= LAG0:
                        flush_one()
                    pend.append((it, jb, pT))
                    if n >= pre_at:
                        pit1, pjb1 = pre_list[n - pre_at]
                        pTp = sbp.tile([128, 2, 512], BF16,
                                       name=f"pT1p_{pit1}_{pjb1}",
                                       tag=f"pT1p_{pit1}_{pjb1}", bufs=1)
                        score(1, pit1, pjb1, pTp)
                        pT1pre[(pit1, pjb1)] = pTp
                for blist in buckets.values():
                    for f in blist:
                        emit_filler(f)
                while pend:
                    flush_one()

        # ---------------- phase B: pair-1 stream + out projection --------
        LAG = 3
        with (
            tc.tile_pool(name="poolB_ps", bufs=1, space="PSUM") as psb,
        ):
            pools["B"] = psb

            def ctx1_tile():
                return psb.tile([128, 2, 512], FP32, name="ctx1", tag="ctx1",
                                bufs=1)

            # it3 is the only exp-paced segment left; everything after
            # it is dense pre-scored AV + output-projection matmul work, so
            # the PE stays busy (and the HAM clock warm) to the very end.
            it = 3
            njb = 16
            ctx1 = ctx1_tile()
            ring = {}
            for jb in range(njb):
                pT = sbp.tile([128, 2, 512], BF16, name="pT1", tag="pT1",
                              bufs=LAG + 2)
                score(1, it, jb, pT)
                ring[jb] = pT
                if jb >= LAG:
                    av(1, it, jb - LAG, ring.pop(jb - LAG), ctx1)
            for j2 in range(njb - LAG, njb):
                av(1, it, j2, ring.pop(j2), ctx1)
            sums_and_drain(3, 1, ctx1, seng=True)
            # it2: 12 AVs woven with it3's output tiles
            ctxp = ps_s_tile()
            bc_normalize(3, 1)
            for jb in range(12):
                av(1, 2, jb, pT1pre[(2, jb)], ctxp)
                if 3 <= jb <= 6:
                    out_proj(12 + (jb - 3), last=True)  # it3 tiles
            sums_and_drain(2, 1, ctxp, seng=True)
            # it1: 8 AVs woven with it2's output tiles
            ctxp = ps_s_tile()
            bc_normalize(2, 1)
            for jb in range(8):
                av(1, 1, jb, pT1pre[(1, jb)], ctxp)
                if 2 <= jb <= 5:
                    out_proj(8 + (jb - 2), last=True)   # it2 tiles
            sums_and_drain(1, 1, ctxp, seng=True)
            # it0: 4 AVs, then it1's output tiles
            ctxp = ps_s_tile()
            bc_normalize(1, 1)
            for jb in range(4):
                av(1, 0, jb, pT1pre[(0, jb)], ctxp)
            for tb in range(4, 8):
                out_proj(tb, last=True)                 # it1 tiles
            sums_and_drain(0, 1, ctxp, seng=True)
            bc_normalize(0, 1)
            for tb in range(0, 4):
                out_proj(tb, last=True)                 # it0 tiles

    if compile:
        nc.compile()
    return nc


_PROGRAM = None


def _get_program():
    global _PROGRAM
    if _PROGRAM is None:
        _PROGRAM = build_program()
    return _PROGRAM


def _consts():
    c = np.zeros((128, 448), ml_dtypes.bfloat16)
    dj = np.arange(128)[:, None]
    di = np.arange(128)[None, :]
    c[:, 0:128] = (dj <= di).astype(ml_dtypes.bfloat16)  # causal triangle
    c[0, 128:192] = 1.0    # sel_h0: ones on cols 0:64
    c[0, 320:384] = 1.0    # sel_h1: ones on cols 64:128
    c[:, 384:448] = 1.0    # ones columns for V_aug
    return c


def make_in_maps(x, Wqkv, Wout):
    in_maps = []
    for core in range(NCORES):
        b, hg = core // (NCORES // B), core % (NCORES // B)
        c0 = hg * HPC * Dh
        blocks = [
            Wqkv[:, c0: c0 + 128],                    # q pair0
            Wqkv[:, D + c0: D + c0 + 128],            # k pair0
            Wqkv[:, c0 + 128: c0 + 256],              # q pair1
            Wqkv[:, D + c0 + 128: D + c0 + 256],      # k pair1
        ]
        wqk_full = np.concatenate(blocks, axis=1).astype(ml_dtypes.bfloat16)
        wv_full = Wqkv[:, 2 * D + c0: 2 * D + c0 + HPC * Dh].astype(
            ml_dtypes.bfloat16)
        in_maps.append({
            "consts": _consts(),
            "xT": np.ascontiguousarray(x[b].T).astype(ml_dtypes.bfloat16),
            "wqk": np.ascontiguousarray(
                wqk_full.reshape(KO, 128, 4 * 128).transpose(1, 0, 2)),
            "wv": np.ascontiguousarray(
                wv_full.reshape(KO, 128, HPC * Dh).transpose(1, 0, 2)),
            "wout": np.ascontiguousarray(
                Wout[c0: c0 + HPC * Dh, :].astype(ml_dtypes.bfloat16)
                .reshape(2, 128, D).transpose(1, 0, 2)),
        })
    return in_maps


def kernel(x, causal_mask, key_padding_mask, Wqkv, bqkv, Wout, bout,
           _trace=False):
    from concourse.bass_utils import run_bass_kernel_spmd

    x = np.asarray(x, dtype=np.float32)
    Wqkv = np.asarray(Wqkv, dtype=np.float32)
    Wout = np.asarray(Wout, dtype=np.float32)
    bqkv = np.asarray(bqkv, dtype=np.float32)
    bout = np.asarray(bout, dtype=np.float32)
    if np.any(np.asarray(key_padding_mask)):
        raise NotImplementedError("key_padding_mask with padded keys")
    if np.any(bqkv):
        raise NotImplementedError("nonzero bqkv")

    nc = _get_program()
    in_maps = make_in_maps(x, Wqkv, Wout)
    res = run_bass_kernel_spmd(nc, in_maps, core_ids=list(range(NCORES)),
                               trace=_trace)
    G = NCORES // B
    outp = np.empty((B, T, D), dtype=np.float32)
    for b in range(B):
        acc = res.results[b * G]["out"].astype(np.float32)
        for hg in range(1, G):
            acc = acc + res.results[b * G + hg]["out"].astype(np.float32)
        outp[b] = acc + bout
    kernel.last_exec_time_ns = res.exec_time_ns
    return outp
